# revision 1
# baseline (speedup 1.0000x reference)
"""Cubic B-spline evaluation (uniform knots) on 8 Trainium2 NeuronCores.

v2: j = 2q + r split.  On segment j the spline is a cubic in v = x - 2q:
  out = HC(v) + r * HD(v),  HC = sum_k c_k[q] v^k,  HD = sum_k d_k[q] v^k
with 32-entry tables c, d (host-derived from coefs).  Table lookups become
step sums over 32 thresholds 1{q >= i} = 1{j >= 2i}, built as a K=5 bf16
matmul over 4 point-slots packed into 128 partitions (32 rows each), an
indicator pass (ScalarE Sign / VectorE is_ge), and a contraction with bf16
hi+lo difference weights.  Coefficient octets stream through DRAM scratch
into pointwise layout; a dual Horner finishes.

Layout (per core, N = 131072 = 4 slots x 32768):
  pointwise: x_pw[p, f] = x[1024 p + f]; p = 32 s + q, q = 2 t + b
  tiles: 64 x 512 cols; chunk t in [0,16) x tau in [0,4); tg = 4t + tau;
         q = tg//2, h = tg%2; unit U = q//2 = t, e = q%2
  g_all[p, cd, k, h, c]: coef k of table cd for point (p, f = 512 h + c)
"""

import sys

sys.path.insert(0, "/opt/trn_rl_repo")

import numpy as np

N_TOTAL = 1_048_576
N_CORES = 8
N = N_TOTAL // N_CORES  # 131072 points per core
P = 128
COLS = N // P  # 1024
TW = 512
NCHUNK = 16
TPC = 4
CH = TPC * TW  # 4096
NSLOT = 4
SLOTN = N // NSLOT  # 32768


def _tables(coefs: np.ndarray):
    import ml_dtypes

    c = np.zeros(67, np.float64)
    c[3:] = np.asarray(coefs, np.float64)
    jj = np.arange(64)
    a0 = (c[jj] + 4 * c[jj + 1] + c[jj + 2]) / 6
    a1 = (c[jj + 2] - c[jj]) / 2
    a2 = (c[jj] - 2 * c[jj + 1] + c[jj + 2]) / 2
    a3 = (c[jj + 3] - c[jj] + 3 * c[jj + 1] - 3 * c[jj + 2]) / 6
    A = np.stack([a0, a1, a2, a3], 1)  # [64, 4] coeffs in u = x - j

    # rebase odd segments to v = u + 1 (v = x - 2q)
    B = A.copy()
    r1 = jj % 2 == 1
    B[r1, 0] = A[r1, 0] - A[r1, 1] + A[r1, 2] - A[r1, 3]
    B[r1, 1] = A[r1, 1] - 2 * A[r1, 2] + 3 * A[r1, 3]
    B[r1, 2] = A[r1, 2] - 3 * A[r1, 3]
    B[r1, 3] = A[r1, 3]
    C = B[0::2]  # [32, 4]
    D = B[1::2] - B[0::2]  # [32, 4]

    # halved step-difference weights (unified sign/{0,2} convention)
    WC = C.copy()
    WC[1:] -= C[:-1]
    WD = D.copy()
    WD[1:] -= D[:-1]
    Wp = np.concatenate([WC, WD], 1) * 0.5  # [32, 8]: col 4 cd + k
    gamma_k = Wp.sum(0).astype(np.float32)  # [8]

    # MM1 lhsT [5, 128]: col m = 32 s + i -> psum = jf_s - thr_i
    w1 = np.zeros((5, 128), np.float64)
    thr = np.empty(32)
    thr[0] = -1.0
    thr[1:] = 2.0 * np.arange(1, 32) - 0.5
    for s in range(4):
        w1[s, 32 * s : 32 * s + 32] = 1.0
        w1[4, 32 * s : 32 * s + 32] = -thr
    # MM2 lhsT [128, 32]: row m = 32 s' + i, col 8 s + 4 cd + k
    w2 = np.zeros((128, 32), np.float64)
    for s in range(4):
        w2[32 * s : 32 * s + 32, 8 * s : 8 * s + 8] = Wp
    bf = ml_dtypes.bfloat16
    w2hi = w2.astype(bf)
    w2lo = (w2 - w2hi.astype(np.float64)).astype(bf)
    return A, w1.astype(bf), (w2hi, w2lo), gamma_k


def _eng_of(t: int, b: int) -> str:
    return "act" if (4 * t + b) % 5 < 3 else "dve"


def _gamma_vec(gamma_k: np.ndarray) -> np.ndarray:
    g = np.zeros((P, 8), np.float32)
    for p in range(P):
        q = p % 32  # q = 2 t + b  (TPC = 4: two pairs per chunk)
        if _eng_of(q // 2, q % 2) == "act":
            g[p] = gamma_k
    return g


_PROG_CACHE: dict = {}


def _build_program():
    import concourse.bacc as bacc
    import concourse.mybir as mybir
    from concourse.tile import TileContext

    f32 = mybir.dt.float32
    bf16 = mybir.dt.bfloat16
    Alu = mybir.AluOpType

    nc = bacc.Bacc("TRN2", debug=False)

    x_dram = nc.dram_tensor("x", [N], f32, kind="ExternalInput")
    w1_dram = nc.dram_tensor("w1", [5, P], bf16, kind="ExternalInput")
    w2hi_dram = nc.dram_tensor("w2hi", [P, 32], bf16, kind="ExternalInput")
    w2lo_dram = nc.dram_tensor("w2lo", [P, 32], bf16, kind="ExternalInput")
    g_dram = nc.dram_tensor("gamma", [P, 8], f32, kind="ExternalInput")
    ones_dram = nc.dram_tensor("ones", [1, CH], bf16, kind="ExternalInput")
    out_dram = nc.dram_tensor("out", [N], f32, kind="ExternalOutput")
    jf_dram = nc.dram_tensor("jf_scratch", [N], bf16, kind="Internal")
    g_dram_s = nc.dram_tensor("g_scratch", [16, 32, 4 * TW], f32, kind="Internal")

    x_view = x_dram.ap().rearrange("(p f) -> p f", p=P)
    out_view = out_dram.ap().rearrange("(p f) -> p f", p=P)

    with TileContext(nc) as tc:
        with (
            tc.tile_pool(name="const", bufs=1) as cpool,
            tc.tile_pool(name="pw", bufs=1) as pw,
            tc.tile_pool(name="tmp", bufs=6) as tmp,
            tc.tile_pool(name="sind", bufs=1) as spool,
            tc.tile_pool(name="gcp", bufs=1) as gcpool,
            tc.tile_pool(name="psum1", bufs=1, space="PSUM") as pp1,
            tc.tile_pool(name="psum2", bufs=1, space="PSUM") as pp2,
        ):
            # ---- constants ----
            w1_sb = cpool.tile([5, P], bf16, tag="w1")
            nc.sync.dma_start(out=w1_sb[:], in_=w1_dram.ap())
            w2hi_sb = cpool.tile([P, 32], bf16, tag="w2hi")
            nc.sync.dma_start(out=w2hi_sb[:], in_=w2hi_dram.ap())
            w2lo_sb = cpool.tile([P, 32], bf16, tag="w2lo")
            nc.sync.dma_start(out=w2lo_sb[:], in_=w2lo_dram.ap())
            gam_sb = cpool.tile([P, 8], f32, tag="gam")
            nc.sync.dma_start(out=gam_sb[:], in_=g_dram.ap())
            j_bufs = []
            for bi in range(3):
                jb = cpool.tile([5, CH], bf16, tag=f"jbuf{bi}", name=f"jbuf{bi}")
                nc.sync.dma_start(out=jb[4:5, :], in_=ones_dram.ap())
                j_bufs.append(jb)

            ps1_bufs = [
                pp1.tile([P, TW], f32, tag=f"s1_{i}", name=f"ps1f{i}")
                for i in range(4)
            ]
            ps2_bufs = [
                pp2.tile([32, 2 * TW], f32, tag=f"s2_{i}", name=f"ps2f{i}")
                for i in range(2)
            ]
            s_bufs = [
                spool.tile([P, TW], bf16, tag=f"sb_{i}", name=f"sbf{i}")
                for i in range(8)
            ]
            gcp_full = [
                gcpool.tile([32, 4 * TW], f32, tag=f"gc_{i}", name=f"gcpf{i}")
                for i in range(2)
            ]

            # dummies: absorb constant-load DMA sems into the PE vector clock
            pdum = ps1_bufs[0]
            nc.tensor.matmul(
                out=pdum[:, 0:8], lhsT=w1_sb[:], rhs=w1_sb[:, 0:8],
                start=True, stop=True,
            )
            nc.tensor.matmul(
                out=pdum[0:32, 0:8], lhsT=w2hi_sb[:], rhs=w2hi_sb[:, 0:8],
                start=True, stop=True,
            )
            nc.tensor.matmul(
                out=pdum[0:32, 0:8], lhsT=w2lo_sb[:], rhs=w2lo_sb[:, 0:8],
                start=True, stop=True,
            )

            # ---- pointwise prep: jf = floor(x), qf = floor(x/2) ----
            x_pw = pw.tile([P, COLS], f32, tag="x")
            nc.sync.dma_start(out=x_pw[:], in_=x_view)
            jf_pw = pw.tile([P, COLS], bf16, tag="jf")
            r_pw = pw.tile([P, COLS], f32, tag="r")
            nc.vector.tensor_scalar(
                r_pw[:], x_pw[:], 8388608.0, -8388608.0, Alu.add, Alu.add
            )
            d_pw = pw.tile([P, COLS], f32, tag="d")
            nc.vector.tensor_tensor(
                out=d_pw[:], in0=r_pw[:], in1=x_pw[:], op=Alu.is_gt
            )
            nc.vector.tensor_tensor(
                out=jf_pw[:], in0=r_pw[:], in1=d_pw[:], op=Alu.subtract
            )
            hx_pw = pw.tile([P, COLS], f32, tag="hx")
            nc.scalar.mul(hx_pw[:], x_pw[:], 0.5)
            t2_pw = pw.tile([P, COLS], f32, tag="t2")
            nc.vector.tensor_scalar(
                t2_pw[:], hx_pw[:], 8388608.0, -8388608.0, Alu.add, Alu.add
            )
            d2_pw = pw.tile([P, COLS], f32, tag="d2")
            nc.vector.tensor_tensor(
                out=d2_pw[:], in0=t2_pw[:], in1=hx_pw[:], op=Alu.is_gt
            )
            qf_pw = pw.tile([P, COLS], f32, tag="qf")
            nc.vector.tensor_tensor(
                out=qf_pw[:], in0=t2_pw[:], in1=d2_pw[:], op=Alu.subtract
            )
            v_pw = pw.tile([P, COLS], f32, tag="v")
            nc.vector.scalar_tensor_tensor(
                v_pw[:], qf_pw[:], -2.0, x_pw[:], Alu.mult, Alu.add
            )
            # r = jf - 2 qf  (0/1)
            rr_pw = pw.tile([P, COLS], f32, tag="rr")
            nc.vector.scalar_tensor_tensor(
                rr_pw[:], qf_pw[:], -2.0, jf_pw[:], Alu.mult, Alu.add
            )

            nc.sync.dma_start(
                out=jf_dram.ap().rearrange("(p f) -> p f", p=P), in_=jf_pw[:]
            )
            jf_view = jf_dram.ap().rearrange(
                "(s t cc) -> s t cc", s=NSLOT, t=NCHUNK
            )

            g_all = pw.tile([P, 2, 4, 2, TW], f32, tag="gall")
            g_view = g_dram_s.ap().rearrange(
                "u m (e h c) -> m u e (h c)", e=2, h=2
            )

            # ---- chunk loop ----
            for t in range(NCHUNK):
                if t == 9:
                    # first half of the units is stored; stream those loads
                    for cd in range(2):
                        for k in range(4):
                            for s in range(4):
                                nc.sync.dma_start(
                                    out=g_all[32 * s : 32 * s + 16, cd, k, :, :],
                                    in_=g_view[8 * s + 4 * cd + k, 0:8],
                                )
                j_pk = j_bufs[t % 3]
                nc.sync.dma_start(out=j_pk[0:4, :], in_=jf_view[:, t])
                # consolidator for the jf-load semaphore
                nc.tensor.matmul(
                    out=ps1_bufs[0][:, 0:8], lhsT=w1_sb[0:4, :],
                    rhs=j_pk[0:4, 0:8], start=True, stop=True,
                )
                for tau in range(TPC):
                    b, h = tau // 2, tau % 2
                    tg = TPC * t + tau
                    q = tg // 2  # = 4 t + b
                    ps1 = ps1_bufs[tg % 4]
                    nc.tensor.matmul(
                        out=ps1[:],
                        lhsT=w1_sb[:],
                        rhs=j_pk[:, tau * TW : (tau + 1) * TW],
                        start=True,
                        stop=True,
                    )
                    s_sb = s_bufs[tg % 8]
                    if _eng_of(t, b) == "act":
                        nc.scalar.sign(s_sb[:], ps1[:])  # {-1, +1}
                    else:
                        nc.vector.tensor_scalar(
                            s_sb[:], ps1[:], 0.0, 2.0, Alu.is_ge, Alu.mult
                        )  # {0, 2}
                    ps2 = ps2_bufs[q % 2]
                    nc.tensor.matmul(
                        out=ps2[:, h * TW : (h + 1) * TW],
                        lhsT=w2hi_sb[:], rhs=s_sb[:],
                        start=True, stop=False,
                    )
                    nc.tensor.matmul(
                        out=ps2[:, h * TW : (h + 1) * TW],
                        lhsT=w2lo_sb[:], rhs=s_sb[:],
                        start=False, stop=True,
                    )
                    if h == 1:
                        gcp = gcp_full[(q // 2) % 2]
                        dstc = gcp[:, (q % 2) * 2 * TW : (q % 2 + 1) * 2 * TW]
                        if (q * 3) % 5 < 3:
                            nc.scalar.copy(out=dstc, in_=ps2[:])
                        else:
                            nc.vector.tensor_copy(out=dstc, in_=ps2[:])
                    if tau % 4 == 3:
                        U = tg // 4
                        nc.gpsimd.dma_start(
                            out=g_dram_s.ap()[U], in_=gcp_full[U % 2][:]
                        )

            # ---- remaining G loads (u >= 8) ----
            for cd in range(2):
                for k in range(4):
                    for s in range(4):
                        nc.sync.dma_start(
                            out=g_all[32 * s + 16 : 32 * s + 32, cd, k, :, :],
                            in_=g_view[8 * s + 4 * cd + k, 8:16],
                        )

            # ---- dual Horner: out = HC(v) + r * HD(v), + gamma on ACT rows --
            v2_pw = pw.tile([P, COLS], f32, tag="v2")
            nc.scalar.square(v2_pw[:], v_pw[:])
            hres = []
            for cd in range(2):
                gk = [
                    g_all[:, cd, k].rearrange("p h c -> p (h c)")
                    for k in range(4)
                ]
                g2c = tmp.tile([P, COLS], f32, tag="ta", name=f"g2c{cd}")
                nc.vector.tensor_scalar(
                    g2c[:], gk[2], gam_sb[:, 4 * cd + 2 : 4 * cd + 3], None,
                    Alu.add,
                )
                g3c = tmp.tile([P, COLS], f32, tag="tb", name=f"g3c{cd}")
                nc.vector.tensor_scalar(
                    g3c[:], gk[3], gam_sb[:, 4 * cd + 3 : 4 * cd + 4], None,
                    Alu.add,
                )
                v1t = tmp.tile([P, COLS], f32, tag="tc", name=f"v1t{cd}")
                nc.vector.tensor_tensor(
                    out=v1t[:], in0=g2c[:], in1=v2_pw[:], op=Alu.mult
                )
                v2t = tmp.tile([P, COLS], f32, tag="td", name=f"v2t{cd}")
                nc.vector.tensor_tensor(
                    out=v2t[:], in0=g3c[:], in1=v2_pw[:], op=Alu.mult
                )
                pacc = tmp.tile([P, COLS], f32, tag="ta", name=f"pacc{cd}")
                nc.vector.scalar_tensor_tensor(
                    pacc[:], v1t[:], gam_sb[:, 4 * cd : 4 * cd + 1], gk[0],
                    Alu.add, Alu.add,
                )
                qacc = tmp.tile([P, COLS], f32, tag="tb", name=f"qacc{cd}")
                nc.vector.scalar_tensor_tensor(
                    qacc[:], v2t[:], gam_sb[:, 4 * cd + 1 : 4 * cd + 2], gk[1],
                    Alu.add, Alu.add,
                )
                v3t = tmp.tile([P, COLS], f32, tag="tc", name=f"v3t{cd}")
                nc.vector.tensor_tensor(
                    out=v3t[:], in0=qacc[:], in1=v_pw[:], op=Alu.mult
                )
                hr = tmp.tile([P, COLS], f32, tag="td", name=f"hr{cd}")
                nc.vector.tensor_tensor(
                    out=hr[:], in0=pacc[:], in1=v3t[:], op=Alu.add
                )
                hres.append(hr)
            rd = tmp.tile([P, COLS], f32, tag="ta", name="rd")
            nc.vector.tensor_tensor(
                out=rd[:], in0=hres[1][:], in1=rr_pw[:], op=Alu.mult
            )
            res = tmp.tile([P, COLS], f32, tag="tb", name="res")
            nc.vector.tensor_tensor(
                out=res[:], in0=hres[0][:], in1=rd[:], op=Alu.add
            )
            nc.sync.dma_start(out=out_view, in_=res[:])

    nc.compile()
    return nc


def get_program():
    if "prog" not in _PROG_CACHE:
        _PROG_CACHE["prog"] = _build_program()
    return _PROG_CACHE["prog"]


def make_in_maps(x: np.ndarray, coefs: np.ndarray):
    import ml_dtypes

    _, w1, (w2hi, w2lo), gamma_k = _tables(coefs)
    gvec = _gamma_vec(gamma_k)
    shards = np.asarray(x, np.float32).reshape(N_CORES, N)
    ones = np.ones((1, CH), ml_dtypes.bfloat16)
    return [
        {
            "x": shards[i].copy(),
            "w1": w1,
            "w2hi": w2hi,
            "w2lo": w2lo,
            "gamma": gvec,
            "ones": ones,
        }
        for i in range(N_CORES)
    ]


def kernel(x, coefs, knot_vector=None, _trace: bool = False):
    from concourse.bass_utils import run_bass_kernel_spmd

    nc = get_program()
    in_maps = make_in_maps(x, coefs)
    res = run_bass_kernel_spmd(nc, in_maps, list(range(N_CORES)), trace=_trace)
    out = np.concatenate([r["out"] for r in res.results])
    if _trace:
        return out, res
    return out



# revision 9
# speedup vs baseline: 1.5730x; 1.5730x over previous
"""Cubic B-spline evaluation (uniform knots) on 8 Trainium2 NeuronCores.

v3: even-knot centering.  c = RNE(xh/2) (xh = fp16(x)), z = x - 2c in
[-1,1]; out = Cz[c](z) + sign(z) * Dz[c](z) with 31-entry tables (c in
[2,32]) derived host-side from coefs.

Per MM tile [128, 512] (4 slots x 512 points): MM1 (fp16) broadcasts the
4 slot rows of xh to 128 partitions; an indicator pass converts psum rows
to step values (Act: Sign with per-partition -thr bias -> {-1,+1}; DVE /
Pool: tensor_scalar add-bias + is_ge -> {0,1}); MM2 (fp16 weights, exact
+-1/0/1 activations) contracts to the 8 per-point table values
(Cz0..3, Dz0..3).  Four tau-tiles stack into one [128, 512] psum via
tile_position, so the psum -> fp16 copy runs on all 128 partitions.
Tables round-trip DRAM in fp16, are re-loaded pointwise, and a dual
fp16 Horner in z finishes.  Output fp16, widened to f32 on host.

Layout (per core, N = 131072): point n = 32768 s + 2048 t + 512 tau + c
lives at pointwise partition p' = 64 (t//8) + 16 s + 2 (t%8) + tau//2,
free f = 512 (tau%2) + c, so each g-load wave (t-half H) fills one
contiguous 64-partition range.  MM tile (t, tau) covers the 4 slots.
"""

import sys

sys.path.insert(0, "/opt/trn_rl_repo")

import numpy as np

N_TOTAL = 1_048_576
N_CORES = 8
N = N_TOTAL // N_CORES  # 131072
P = 128
COLS = N // P  # 1024
TW = 512
NCHUNK = 16
TPC = 4
M32 = 12582912.0  # 1.5 * 2**23

# engine schedule: 64 indicator tiles (a=Act, d=DVE, p=Pool), 16 copies
_SIGN_PAT = ("aadadaad" * 8)[:64]  # 40 a, 24 d
_COPY_PAT = ("adda" * 4)[:16]      # 8 a, 8 d


def _seg_polys(coefs):
    c = np.zeros(67)
    c[3:] = np.asarray(coefs, np.float64)
    jj = np.arange(64)
    a0 = (c[jj] + 4 * c[jj + 1] + c[jj + 2]) / 6
    a1 = (c[jj + 2] - c[jj]) / 2
    a2 = (c[jj] - 2 * c[jj + 1] + c[jj + 2]) / 2
    a3 = (c[jj + 3] - c[jj] + 3 * c[jj + 1] - 3 * c[jj + 2]) / 6
    return np.stack([a0, a1, a2, a3], 1)  # [64, 4] in u = x - j


def _shift_poly(P_, d):
    from math import comb

    Q = np.zeros_like(P_)
    for k in range(4):
        for m in range(k + 1):
            Q[:, m] += P_[:, k] * comb(k, m) * d ** (k - m)
    return Q


def _tables(coefs):
    """MM2 step-sum weights [128, 32] (both conventions) + sign biases."""
    A = _seg_polys(coefs)
    Ez = np.zeros((33, 4))
    Oz = np.zeros((33, 4))
    for c in range(2, 33):
        Ez[c] = _shift_poly(A[2 * c - 1 : 2 * c], 1.0)[0]  # segment 2c-1, z<0
        Oz[c] = A[2 * c] if 2 * c < 64 else Ez[c]          # segment 2c,  z>=0
    Cz = (Ez + Oz) / 2
    Dz = (Oz - Ez) / 2

    def stepw(T):  # rows r: 0 base (c=2), 1 spare, r>=2: 1{c >= r+1}
        W = np.zeros((32, 4))
        W[0] = T[2]
        W[2:] = T[3:] - T[2:-1]
        return W

    WC, WD = stepw(Cz), stepw(Dz)
    WCa = WC / 2
    WCa[0] = WC[0] + WC[2:].sum(0) / 2
    WDa = WD / 2
    WDa[0] = WD[0] + WD[2:].sum(0) / 2

    def pack(WCx, WDx):  # lhsT [128, 32]: in-row 32 s + r -> out 8 s + v
        W = np.zeros((128, 32), np.float16)
        for s in range(4):
            W[32 * s : 32 * s + 32, 8 * s : 8 * s + 4] = WCx.astype(np.float16)
            W[32 * s : 32 * s + 32, 8 * s + 4 : 8 * s + 8] = WDx.astype(np.float16)
        return W

    thr = np.zeros(32, np.float64)
    thr[0] = thr[1] = -1e5
    for r in range(2, 32):
        i = r + 1
        eps = 2.0 ** -10
        thr[r] = (2 * i - 1) - eps if i % 2 == 0 else (2 * i - 1) + eps
    bias = np.tile(-thr, 4).astype(np.float32).reshape(128, 1)
    return pack(WCa, WDa), pack(WC, WD), bias


_PROG_CACHE: dict = {}


def _build_program():
    import concourse.bacc as bacc
    import concourse.mybir as mybir
    from concourse.tile import TileContext

    f32 = mybir.dt.float32
    fp16 = mybir.dt.float16
    Alu = mybir.AluOpType
    Act = mybir.ActivationFunctionType

    nc = bacc.Bacc("TRN2", debug=False)

    x_dram = nc.dram_tensor("x", [N], f32, kind="ExternalInput")
    wsgn_dram = nc.dram_tensor("wsgn", [P, 32], fp16, kind="ExternalInput")
    wstp_dram = nc.dram_tensor("wstp", [P, 32], fp16, kind="ExternalInput")
    bias_dram = nc.dram_tensor("bias", [P, 1], f32, kind="ExternalInput")
    w1_dram = nc.dram_tensor("w1", [4, P], fp16, kind="ExternalInput")
    out_dram = nc.dram_tensor("out", [N], fp16, kind="ExternalOutput")
    xh_dram = nc.dram_tensor("xh_scratch", [N], fp16, kind="Internal")
    # g scratch [t, row, col]: row = 64 tauh + 32 taul + 8 s + v
    g_dram = nc.dram_tensor("g_scratch", [NCHUNK, P, TW], fp16, kind="Internal")

    def n_view(t1d):
        # element order (H, s, tp, tauh, taul, c) == pointwise (p', f) order
        return t1d.ap().rearrange(
            "(s H tp tauh taul c) -> H s tp tauh taul c",
            s=4, H=2, tp=8, tauh=2, taul=2,
        )

    # per-half views: [H][s, tp, tauh, taul, c] -> merges to 3 dims for DMA
    # g load view: dims [v, s, t, tauh, taul, c]
    g_in_view = g_dram.ap().rearrange(
        "t (tauh taul s v) c -> v s t tauh taul c", tauh=2, taul=2, s=4
    )

    with TileContext(nc) as tc:
        with (
            tc.tile_pool(name="const", bufs=1) as cpool,
            tc.tile_pool(name="pw", bufs=1) as pw,
            tc.tile_pool(name="sind", bufs=1) as spool,
            tc.tile_pool(name="gbig", bufs=1) as gpool,
            tc.tile_pool(name="gall", bufs=1) as gapool,
            tc.tile_pool(name="htmp", bufs=1) as hpool,
            tc.tile_pool(name="psum1", bufs=1, space="PSUM") as pp1,
            tc.tile_pool(name="psum2", bufs=1, space="PSUM") as pp2,
        ):
            # ---- constants ----
            w1_sb = cpool.tile([4, P], fp16, tag="w1")
            nc.sync.dma_start(out=w1_sb[:], in_=w1_dram.ap())
            wsgn_sb = cpool.tile([P, 32], fp16, tag="wsgn")
            nc.sync.dma_start(out=wsgn_sb[:], in_=wsgn_dram.ap())
            wstp_sb = cpool.tile([P, 32], fp16, tag="wstp")
            nc.sync.dma_start(out=wstp_sb[:], in_=wstp_dram.ap())
            bias_sb = cpool.tile([P, 1], f32, tag="bias")
            nc.sync.dma_start(out=bias_sb[:], in_=bias_dram.ap())

            # ---- pointwise prep ----
            x_pw = pw.tile([P, COLS], f32, tag="x")
            xv = n_view(x_dram)
            nc.sync.dma_start(out=x_pw[0:64, :], in_=xv[0])
            nc.sync.dma_start(out=x_pw[64:128, :], in_=xv[1])
            xh_pw = pw.tile([P, COLS], fp16, tag="xh")
            nc.scalar.copy(out=xh_pw[:], in_=x_pw[:])
            xhv = n_view(xh_dram)
            nc.sync.dma_start(out=xhv[0], in_=xh_pw[0:64, :])
            nc.sync.dma_start(out=xhv[1], in_=xh_pw[64:128, :])
            xh_mm = pw.tile([4, N // 4], fp16, tag="xhmm")
            nc.sync.dma_start(
                out=xh_mm[:], in_=xh_dram.ap().rearrange("(s f) -> s f", s=4)
            )

            t_r = pw.tile([P, COLS], f32, tag="tr")
            nc.scalar.activation(t_r[:], xh_pw[:], Act.Copy, bias=M32, scale=0.5)
            qb = pw.tile([P, COLS], f32, tag="qb")
            nc.gpsimd.tensor_scalar(
                qb[:], t_r[:], M32, 2.0, Alu.subtract, Alu.mult
            )
            z_pw = pw.tile([P, COLS], f32, tag="z")
            nc.gpsimd.tensor_tensor(
                out=z_pw[:], in0=x_pw[:], in1=qb[:], op=Alu.subtract
            )
            rp_pw = pw.tile([P, COLS], fp16, tag="rp")
            nc.scalar.sign(rp_pw[:], z_pw[:])
            zh_pw = pw.tile([P, COLS], fp16, tag="zh")
            nc.scalar.copy(out=zh_pw[:], in_=z_pw[:])
            z2_pw = pw.tile([P, COLS], fp16, tag="z2")
            nc.scalar.square(z2_pw[:], z_pw[:])

            # ---- indicator + table matmuls ----
            s_bufs = [
                spool.tile([P, TW], fp16, tag=f"s{i}", name=f"sbf{i}")
                for i in range(6)
            ]
            ps1_bufs = [
                pp1.tile([P, TW], f32, tag=f"p1_{i}", name=f"ps1f{i}")
                for i in range(4)
            ]
            ps2_bufs = [
                pp2.tile([P, TW], f32, tag=f"p2_{i}", name=f"ps2f{i}")
                for i in range(2)
            ]
            gbig = [
                gpool.tile([P, 4 * TW], fp16, tag=f"gb{i}", name=f"gbig{i}")
                for i in range(2)
            ]
            g_all = [
                gapool.tile([P, COLS], fp16, tag=f"ga{v}", name=f"gall{v}")
                for v in range(8)
            ]

            def load_g_half(v, H):
                # p' = 64 H + 16 s + 2 t' + tauh: wave H = contiguous 64 rows
                nc.sync.dma_start(
                    out=g_all[v][64 * H : 64 * H + 64, :],
                    in_=g_in_view[v][:, 8 * H : 8 * H + 8],
                )

            for t in range(NCHUNK):
                ps2 = ps2_bufs[t % 2]
                for tau in range(TPC):
                    tile_id = TPC * t + tau
                    eng = _SIGN_PAT[tile_id]
                    ps1 = ps1_bufs[tile_id % 4]
                    nc.tensor.matmul(
                        out=ps1[:],
                        lhsT=w1_sb[:],
                        rhs=xh_mm[:, 2048 * t + TW * tau : 2048 * t + TW * (tau + 1)],
                        start=True,
                        stop=True,
                    )
                    s_sb = s_bufs[tile_id % 6]
                    if eng == "a":
                        nc.scalar.sign(s_sb[:], ps1[:], bias=bias_sb[:, 0:1])
                        w2 = wsgn_sb
                    else:
                        nc.vector.tensor_scalar(
                            s_sb[:], ps1[:], bias_sb[:, 0:1], 0.0,
                            Alu.add, Alu.is_ge,
                        )
                        w2 = wstp_sb
                    nc.tensor.matmul(
                        out=ps2[32 * tau : 32 * tau + 32, :],
                        lhsT=w2[:],
                        rhs=s_sb[:],
                        start=True,
                        stop=True,
                        tile_position=(0, 32 * tau),
                    )
                # psum chunk -> fp16 staging (one op, 128 partitions)
                gdst = gbig[(t // 4) % 2][:, TW * (t % 4) : TW * (t % 4 + 1)]
                ceng = _COPY_PAT[t]
                if ceng == "a":
                    nc.scalar.copy(out=gdst, in_=ps2[:])
                else:
                    nc.vector.tensor_copy(out=gdst, in_=ps2[:])
                if t % 4 == 3:
                    G = t // 4
                    nc.sync.dma_start(
                        out=g_dram.ap()[4 * G : 4 * G + 4].rearrange(
                            "tt p c -> p tt c"
                        ),
                        in_=gbig[G % 2][:].rearrange("p (tt c) -> p tt c", tt=4),
                    )
                if t == 8:
                    for v in range(8):
                        load_g_half(v, 0)

            for v in range(8):
                load_g_half(v, 1)

            # ---- dual fp16 Horner in z ----
            hr = []
            for cd in range(2):
                g0, g1, g2, g3 = (g_all[4 * cd + k] for k in range(4))
                m1 = hpool.tile([P, COLS], fp16, tag=f"m1{cd}", name=f"m1_{cd}")
                nc.vector.tensor_tensor(
                    out=m1[:], in0=g1[:], in1=zh_pw[:], op=Alu.mult
                )
                m2 = hpool.tile([P, COLS], fp16, tag=f"m2{cd}", name=f"m2_{cd}")
                nc.gpsimd.tensor_tensor(
                    out=m2[:], in0=g3[:], in1=zh_pw[:], op=Alu.mult
                )
                e1 = hpool.tile([P, COLS], fp16, tag=f"e1{cd}", name=f"e1_{cd}")
                nc.vector.tensor_tensor(
                    out=e1[:], in0=g0[:], in1=m1[:], op=Alu.add
                )
                e2 = hpool.tile([P, COLS], fp16, tag=f"e2{cd}", name=f"e2_{cd}")
                nc.gpsimd.tensor_tensor(
                    out=e2[:], in0=g2[:], in1=m2[:], op=Alu.add
                )
                m3 = hpool.tile([P, COLS], fp16, tag=f"m3{cd}", name=f"m3_{cd}")
                nc.gpsimd.tensor_tensor(
                    out=m3[:], in0=e2[:], in1=z2_pw[:], op=Alu.mult
                )
                h = hpool.tile([P, COLS], fp16, tag=f"h{cd}", name=f"h_{cd}")
                nc.vector.tensor_tensor(
                    out=h[:], in0=e1[:], in1=m3[:], op=Alu.add
                )
                hr.append(h)
            rd = hpool.tile([P, COLS], fp16, tag="rd", name="rd")
            nc.vector.tensor_tensor(
                out=rd[:], in0=hr[1][:], in1=rp_pw[:], op=Alu.mult
            )
            res = hpool.tile([P, COLS], fp16, tag="res", name="res")
            nc.vector.tensor_tensor(
                out=res[:], in0=hr[0][:], in1=rd[:], op=Alu.add
            )
            ov = n_view(out_dram)
            nc.sync.dma_start(out=ov[0], in_=res[0:64, :])
            nc.sync.dma_start(out=ov[1], in_=res[64:128, :])

    nc.compile()
    return nc


def get_program():
    if "prog" not in _PROG_CACHE:
        _PROG_CACHE["prog"] = _build_program()
    return _PROG_CACHE["prog"]


def make_in_maps(x: np.ndarray, coefs: np.ndarray):
    w_sgn, w_stp, bias = _tables(coefs)
    w1 = np.zeros((4, P), np.float16)
    for s in range(4):
        w1[s, 32 * s : 32 * s + 32] = 1.0
    shards = np.asarray(x, np.float32).reshape(N_CORES, N)
    return [
        {
            "x": shards[i].copy(),
            "wsgn": w_sgn,
            "wstp": w_stp,
            "bias": bias,
            "w1": w1,
        }
        for i in range(N_CORES)
    ]


def kernel(x, coefs, knot_vector=None, _trace: bool = False):
    from concourse.bass_utils import run_bass_kernel_spmd

    nc = get_program()
    in_maps = make_in_maps(x, coefs)
    res = run_bass_kernel_spmd(nc, in_maps, list(range(N_CORES)), trace=_trace)
    out = np.concatenate([r["out"] for r in res.results]).astype(np.float32)
    if _trace:
        return out, res
    return out


# revision 10
# speedup vs baseline: 1.8218x; 1.1582x over previous
"""Cubic B-spline evaluation (uniform knots) on 8 Trainium2 NeuronCores.

v4: even-knot centering + tau-low phasing.  c = RNE(xh/2) (xh = fp16(x)),
z = x - 2c in [-1,1]; out = Cz[c](z) + sign(z) * Dz[c](z) with 31-entry
tables (c in [2,32]) derived host-side from coefs.

Per MM tile [128, 512] (4 slots x 512 points): MM1 (fp16) broadcasts the
4 slot rows of xh to 128 partitions; an indicator pass converts psum rows
to step values (Act: Sign with per-partition -thr bias -> {-1,+1}; DVE:
tensor_scalar add-bias + is_ge -> {0,1}); MM2 (fp16 weights, exact
activations) contracts to the 8 per-point table values.  Four tiles
(parity, tauh) stack into one [128, 512] psum via tile_position so the
psum -> fp16 copy uses all 128 partitions.  Work is phased by
taul = tau % 2: each phase stores its table halves to DRAM, reloads them
pointwise, and runs its half of the dual fp16 Horner, overlapping the
other phase's matmuls.  Output fp16, widened to f32 on host.

Layout (per core, N = 131072): point n = 32768 s + 2048 t + 512 tau + c
with t = 8 Hu + 2 ul + parity, tau = 2 tauh + taul lives at pointwise
partition p' = 64 Hu + 16 ul + 8 parity + 4 tauh + s, free
f = 512 taul + c.  All permutation lives in the DRAM-side DMA views.
"""

import sys

sys.path.insert(0, "/opt/trn_rl_repo")

import numpy as np

N_TOTAL = 1_048_576
N_CORES = 8
N = N_TOTAL // N_CORES  # 131072
P = 128
COLS = N // P  # 1024
TW = 512
M32 = 12582912.0  # 1.5 * 2**23

# engine schedule: 64 indicator tiles (a=Act, d=DVE) in emission order
# (phase A tiles 0..31, phase B tiles 32..63), 16 psum->fp16 copies
_SIGN_PAT = ("ddadadad" * 2 + "adadadad" * 2) + ("adaadada" * 2 + "aadadada" * 2)
_COPY_PAT = "adadadad" + "aadaadad"


def _seg_polys(coefs):
    c = np.zeros(67)
    c[3:] = np.asarray(coefs, np.float64)
    jj = np.arange(64)
    a0 = (c[jj] + 4 * c[jj + 1] + c[jj + 2]) / 6
    a1 = (c[jj + 2] - c[jj]) / 2
    a2 = (c[jj] - 2 * c[jj + 1] + c[jj + 2]) / 2
    a3 = (c[jj + 3] - c[jj] + 3 * c[jj + 1] - 3 * c[jj + 2]) / 6
    return np.stack([a0, a1, a2, a3], 1)  # [64, 4] in u = x - j


def _shift_poly(P_, d):
    from math import comb

    Q = np.zeros_like(P_)
    for k in range(4):
        for m in range(k + 1):
            Q[:, m] += P_[:, k] * comb(k, m) * d ** (k - m)
    return Q


def _tables(coefs):
    """MM2 step-sum weights [128, 32] (both conventions) + sign biases."""
    A = _seg_polys(coefs)
    Ez = np.zeros((33, 4))
    Oz = np.zeros((33, 4))
    for c in range(2, 33):
        Ez[c] = _shift_poly(A[2 * c - 1 : 2 * c], 1.0)[0]  # segment 2c-1, z<0
        Oz[c] = A[2 * c] if 2 * c < 64 else Ez[c]          # segment 2c,  z>=0
    Cz = (Ez + Oz) / 2
    Dz = (Oz - Ez) / 2

    def stepw(T):  # rows r: 0 base (c=2), 1 spare, r>=2: 1{c >= r+1}
        W = np.zeros((32, 4))
        W[0] = T[2]
        W[2:] = T[3:] - T[2:-1]
        return W

    WC, WD = stepw(Cz), stepw(Dz)
    WCa = WC / 2
    WCa[0] = WC[0] + WC[2:].sum(0) / 2
    WDa = WD / 2
    WDa[0] = WD[0] + WD[2:].sum(0) / 2

    def pack(WCx, WDx):  # lhsT [128, 32]: in-row 32 s + r -> out 8 s + v
        W = np.zeros((128, 32), np.float16)
        for s in range(4):
            W[32 * s : 32 * s + 32, 8 * s : 8 * s + 4] = WCx.astype(np.float16)
            W[32 * s : 32 * s + 32, 8 * s + 4 : 8 * s + 8] = WDx.astype(np.float16)
        return W

    thr = np.zeros(32, np.float64)
    thr[0] = thr[1] = -1e5
    for r in range(2, 32):
        i = r + 1
        eps = 2.0 ** -10
        thr[r] = (2 * i - 1) - eps if i % 2 == 0 else (2 * i - 1) + eps
    bias = np.tile(-thr, 4).astype(np.float32).reshape(128, 1)
    return pack(WCa, WDa), pack(WC, WD), bias


_PROG_CACHE: dict = {}


def _build_program():
    import concourse.bacc as bacc
    import concourse.mybir as mybir
    from concourse.tile import TileContext

    f32 = mybir.dt.float32
    fp16 = mybir.dt.float16
    Alu = mybir.AluOpType
    Act = mybir.ActivationFunctionType

    nc = bacc.Bacc("TRN2", debug=False)

    x_dram = nc.dram_tensor("x", [N], f32, kind="ExternalInput")
    wsgn_dram = nc.dram_tensor("wsgn", [P, 32], fp16, kind="ExternalInput")
    wstp_dram = nc.dram_tensor("wstp", [P, 32], fp16, kind="ExternalInput")
    bias_dram = nc.dram_tensor("bias", [P, 1], f32, kind="ExternalInput")
    w1_dram = nc.dram_tensor("w1", [4, P], fp16, kind="ExternalInput")
    out_dram = nc.dram_tensor("out", [N], fp16, kind="ExternalOutput")
    xh_dram = nc.dram_tensor("xh_scratch", [N], fp16, kind="Internal")
    # g scratch [taul, Hu, ul, row, col]: row = 64 parity + 32 tauh + 8 s + v
    g_dram = nc.dram_tensor("g_scratch", [2, 2, 4, P, TW], fp16, kind="Internal")

    def half_view(t1d):
        # [Hu][(ul parity tauh), s, (taul c)]: pointwise half in 3 DMA dims
        return t1d.ap().rearrange(
            "(s Hu ul parity tauh taul c) -> Hu (ul parity tauh) s (taul c)",
            s=4, Hu=2, ul=4, parity=2, tauh=2, taul=2,
        )

    def taul_view(t1d):
        # [taul][(Hu ul parity tauh), s, c]: pointwise f-half in 3 DMA dims
        return t1d.ap().rearrange(
            "(s Hu ul parity tauh taul c) -> taul (Hu ul parity tauh) s c",
            s=4, Hu=2, ul=4, parity=2, tauh=2, taul=2,
        )

    # g load view [taul, v]: merges to [[4096, 128], [1, 512]]
    g_in_view = g_dram.ap().rearrange(
        "tl Hu ul (parity tauh s v) c -> tl v Hu ul parity tauh s c",
        parity=2, tauh=2, s=4,
    )

    with TileContext(nc) as tc:
        with (
            tc.tile_pool(name="const", bufs=1) as cpool,
            tc.tile_pool(name="pw", bufs=1) as pw,
            tc.tile_pool(name="sind", bufs=1) as spool,
            tc.tile_pool(name="gbig", bufs=1) as gpool,
            tc.tile_pool(name="gall", bufs=1) as gapool,
            tc.tile_pool(name="htmp", bufs=1) as hpool,
            tc.tile_pool(name="psum1", bufs=1, space="PSUM") as pp1,
            tc.tile_pool(name="psum2", bufs=1, space="PSUM") as pp2,
        ):
            # ---- constants (Pool SWDGE; off the SP/Act queues) ----
            w1_sb = cpool.tile([4, P], fp16, tag="w1")
            nc.gpsimd.dma_start(out=w1_sb[:], in_=w1_dram.ap())
            wsgn_sb = cpool.tile([P, 32], fp16, tag="wsgn")
            nc.gpsimd.dma_start(out=wsgn_sb[:], in_=wsgn_dram.ap())
            wstp_sb = cpool.tile([P, 32], fp16, tag="wstp")
            nc.gpsimd.dma_start(out=wstp_sb[:], in_=wstp_dram.ap())
            bias_sb = cpool.tile([P, 1], f32, tag="bias")
            nc.gpsimd.dma_start(out=bias_sb[:], in_=bias_dram.ap())

            # ---- pointwise prep (half-pipelined startup) ----
            x_pw = pw.tile([P, COLS], f32, tag="x")
            xh_pw = pw.tile([P, COLS], fp16, tag="xh")
            xh_mm = pw.tile([4, N // 4], fp16, tag="xhmm")
            warm = pw.tile([P, 1], fp16, tag="warm")
            xv = half_view(x_dram)
            xhv = half_view(xh_dram)
            xhmm_in = xh_dram.ap().rearrange("(s f) -> s f", s=4)
            # warm the Act Sign table set during the first x DMA
            nc.scalar.sign(warm[:], bias_sb[:, 0:1])
            for H in (0, 1):
                pr = slice(64 * H, 64 * H + 64)
                nc.sync.dma_start(out=x_pw[pr, :], in_=xv[H])
                nc.scalar.copy(out=xh_pw[pr, :], in_=x_pw[pr, :])
                nc.sync.dma_start(out=xhv[H], in_=xh_pw[pr, :])
                nc.sync.dma_start(
                    out=xh_mm[:, 16384 * H : 16384 * H + 16384],
                    in_=xhmm_in[:, 16384 * H : 16384 * H + 16384],
                )

            t_r = pw.tile([P, COLS], f32, tag="tr")
            nc.scalar.activation(t_r[:], xh_pw[:], Act.Copy, bias=M32, scale=0.5)
            qb = pw.tile([P, COLS], f32, tag="qb")
            nc.gpsimd.tensor_scalar(
                qb[:], t_r[:], M32, 2.0, Alu.subtract, Alu.mult
            )
            z_pw = pw.tile([P, COLS], f32, tag="z")
            nc.gpsimd.tensor_tensor(
                out=z_pw[:], in0=x_pw[:], in1=qb[:], op=Alu.subtract
            )
            rp_pw = pw.tile([P, COLS], fp16, tag="rp")
            nc.scalar.sign(rp_pw[:], z_pw[:])
            zh_pw = pw.tile([P, COLS], fp16, tag="zh")
            nc.vector.tensor_copy(out=zh_pw[:], in_=z_pw[:])
            z2_pw = pw.tile([P, COLS], fp16, tag="z2")
            nc.scalar.square(z2_pw[:], z_pw[:])

            # ---- tiles ----
            s_bufs = [
                spool.tile([P, TW], fp16, tag=f"s{i}", name=f"sbf{i}")
                for i in range(6)
            ]
            ps1_bufs = [
                pp1.tile([P, TW], f32, tag=f"p1_{i}", name=f"ps1f{i}")
                for i in range(4)
            ]
            ps2_bufs = [
                pp2.tile([P, TW], f32, tag=f"p2_{i}", name=f"ps2f{i}")
                for i in range(2)
            ]
            gbig = [
                gpool.tile([P, 4 * TW], fp16, tag=f"gb{i}", name=f"gbig{i}")
                for i in range(2)
            ]
            # per-half pointwise table tiles: [taul][v] -> [128, 512]
            g_half = [
                [
                    gapool.tile([P, TW], fp16, tag=f"g{tl}_{v}", name=f"g{tl}_{v}")
                    for v in range(8)
                ]
                for tl in range(2)
            ]

            res = hpool.tile([P, COLS], fp16, tag="res", name="res")
            ov = taul_view(out_dram)

            def horner_half(tl):
                fs = slice(TW * tl, TW * tl + TW)
                ga = g_half[tl]
                hr = []
                for cd in range(2):
                    g0, g1, g2, g3 = (ga[4 * cd + k] for k in range(4))
                    m1 = hpool.tile([P, TW], fp16, tag=f"m1{cd}", name=f"m1_{cd}{tl}")
                    nc.vector.tensor_tensor(
                        out=m1[:], in0=g1[:], in1=zh_pw[:, fs], op=Alu.mult
                    )
                    m2 = hpool.tile([P, TW], fp16, tag=f"m2{cd}", name=f"m2_{cd}{tl}")
                    nc.vector.tensor_tensor(
                        out=m2[:], in0=g3[:], in1=zh_pw[:, fs], op=Alu.mult
                    )
                    e1 = hpool.tile([P, TW], fp16, tag=f"e1{cd}", name=f"e1_{cd}{tl}")
                    nc.vector.tensor_tensor(
                        out=e1[:], in0=g0[:], in1=m1[:], op=Alu.add
                    )
                    e2 = hpool.tile([P, TW], fp16, tag=f"e2{cd}", name=f"e2_{cd}{tl}")
                    nc.vector.tensor_tensor(
                        out=e2[:], in0=g2[:], in1=m2[:], op=Alu.add
                    )
                    m3 = hpool.tile([P, TW], fp16, tag=f"m3{cd}", name=f"m3_{cd}{tl}")
                    nc.vector.tensor_tensor(
                        out=m3[:], in0=e2[:], in1=z2_pw[:, fs], op=Alu.mult
                    )
                    h = hpool.tile([P, TW], fp16, tag=f"h{cd}", name=f"h_{cd}{tl}")
                    nc.vector.tensor_tensor(
                        out=h[:], in0=e1[:], in1=m3[:], op=Alu.add
                    )
                    hr.append(h)
                rd = hpool.tile([P, TW], fp16, tag="rd", name=f"rd{tl}")
                nc.vector.tensor_tensor(
                    out=rd[:], in0=hr[1][:], in1=rp_pw[:, fs], op=Alu.mult
                )
                nc.vector.tensor_tensor(
                    out=res[:, fs], in0=hr[0][:], in1=rd[:], op=Alu.add
                )
                nc.sync.dma_start(out=ov[tl], in_=res[:, fs])

            # ---- phased matmul pipeline ----
            tile_i = 0
            copy_i = 0
            for tl in (0, 1):
                for Hu in (0, 1):
                    gb = gbig[Hu]
                    for ul in range(4):
                        ps2 = ps2_bufs[ul % 2]
                        for parity in (0, 1):
                            for tauh in (0, 1):
                                t = 8 * Hu + 2 * ul + parity
                                tau = 2 * tauh + tl
                                blk = 2 * parity + tauh
                                eng = _SIGN_PAT[tile_i]
                                tile_i += 1
                                ps1 = ps1_bufs[(4 * ul + blk) % 4]
                                nc.tensor.matmul(
                                    out=ps1[:],
                                    lhsT=w1_sb[:],
                                    rhs=xh_mm[
                                        :,
                                        2048 * t + TW * tau : 2048 * t
                                        + TW * (tau + 1),
                                    ],
                                    start=True,
                                    stop=True,
                                )
                                s_sb = s_bufs[tile_i % 6]
                                if eng == "a":
                                    nc.scalar.sign(
                                        s_sb[:], ps1[:], bias=bias_sb[:, 0:1]
                                    )
                                    w2 = wsgn_sb
                                else:
                                    nc.vector.tensor_scalar(
                                        s_sb[:], ps1[:], bias_sb[:, 0:1], 0.0,
                                        Alu.add, Alu.is_ge,
                                    )
                                    w2 = wstp_sb
                                nc.tensor.matmul(
                                    out=ps2[32 * blk : 32 * blk + 32, :],
                                    lhsT=w2[:],
                                    rhs=s_sb[:],
                                    start=True,
                                    stop=True,
                                    tile_position=(0, 32 * blk),
                                )
                        gdst = gb[:, TW * ul : TW * ul + TW]
                        if _COPY_PAT[copy_i] == "a":
                            nc.scalar.copy(out=gdst, in_=ps2[:])
                        else:
                            nc.vector.tensor_copy(out=gdst, in_=ps2[:])
                        copy_i += 1
                    # store this (tl, Hu) group via Pool SWDGE
                    nc.gpsimd.dma_start(
                        out=g_dram.ap()[tl, Hu].rearrange("ul p c -> p ul c"),
                        in_=gb[:].rearrange("p (ul c) -> p ul c", ul=4),
                    )
                # wave loads for this taul (split across SP and Pool queues)
                for v in range(8):
                    e = nc.sync if v % 2 == 0 else nc.gpsimd
                    e.dma_start(out=g_half[tl][v][:], in_=g_in_view[tl, v])
                horner_half(tl)

    nc.compile()
    return nc


def get_program():
    if "prog" not in _PROG_CACHE:
        _PROG_CACHE["prog"] = _build_program()
    return _PROG_CACHE["prog"]


def make_in_maps(x: np.ndarray, coefs: np.ndarray):
    w_sgn, w_stp, bias = _tables(coefs)
    w1 = np.zeros((4, P), np.float16)
    for s in range(4):
        w1[s, 32 * s : 32 * s + 32] = 1.0
    shards = np.asarray(x, np.float32).reshape(N_CORES, N)
    return [
        {
            "x": shards[i].copy(),
            "wsgn": w_sgn,
            "wstp": w_stp,
            "bias": bias,
            "w1": w1,
        }
        for i in range(N_CORES)
    ]


def kernel(x, coefs, knot_vector=None, _trace: bool = False):
    from concourse.bass_utils import run_bass_kernel_spmd

    nc = get_program()
    in_maps = make_in_maps(x, coefs)
    res = run_bass_kernel_spmd(nc, in_maps, list(range(N_CORES)), trace=_trace)
    out = np.concatenate([r["out"] for r in res.results]).astype(np.float32)
    if _trace:
        return out, res
    return out
